# revision 20
# baseline (speedup 1.0000x reference)
"""Trainium2 Bass kernel for a LLaMA-style causal attention block.

Sharding (8 NeuronCores, one trn2 chip):
  - Tensor-parallel over heads: core c owns heads [4c, 4c+4) -> wq/wk/wv column
    slices [4096, 512]; computes qT/kT/v + RoPE + causal attention for its heads.
  - attnT [512, 2048] (bf16) is AllGather'd per sq quarter -> each core computes
    out[:, 512c:512c+512] = attn @ wo_cols.  Host concatenates column slices.

Layout trick: everything is computed transposed ([head_dim, seq]) so no
on-device transposes are needed:
  qT/kT = w_h.T @ xT      (xT host-pretransposed)
  scoresT[sk, sq] = kT_tile.T @ qT
  attnT[hd, sq] = v_tile.T @ expT
  out[sq, cols] = attnT_full_tile.T @ wo_tile
exp() needs no max-subtraction: scores are O(1) by construction.

v2 structure (vs baseline):
  - wq/wk/wv resident in SBUF, loaded ONCE on three parallel DMA queues
    (baseline re-streamed 12.6MB of weights per strip -> QKV DMA starvation,
    PE idle gaps -> HAM K=4/8 re-throttles).
  - Each strip = three 4-bank passes: A={q(h0),q(h1),k(h0),k(h1)},
    B={q(h2),q(h3),k(h2),k(h3)}, C={v}; attention of the previous strip runs
    as a fourth pass.  At most 8 PSUM banks live, each bank has ~30us of
    evacuation slack -> PE never waits on PSUM.
  - softmax denominator: exp blocks accumulated on DVE into an f32 tile,
    then ONE ones-matmul per (head, quarter) (baseline: a [1,n] matmul per
    block = 69k wasted PE cycles + 160 LDWEIGHTS).
  - 1/den via reciprocal_approx_fast (5x faster than InstReciprocal) and the
    attn PSUM bank is held only until one DVE multiply after broadcast.
  - output stored bf16 (host upcasts), halving the store tail.

Compute dtype bf16 (f32 PSUM accumulation), I/O f32.
"""

import math
import os
import sys

for _p in ("/opt/trn_rl_repo",):
    if os.path.isdir(_p) and _p not in sys.path:
        sys.path.insert(0, _p)

import numpy as np
import ml_dtypes

N_CORES = 8
B, S, D, H = 1, 2048, 4096, 32
HD = D // H          # 128
HPC = H // N_CORES   # 4 heads per core
CW = D // N_CORES    # 512 columns per core
NK = D // 128        # 32 contraction tiles
SQT = 512            # sq tile width
NSQ = S // SQT       # 4
SCALE = 1.0 / math.sqrt(HD)

_CACHE = {}
LAST_RESULT = None   # test harness reads exec_time_ns from here


def _build():
    import concourse.mybir as mybir
    import concourse.tile as tile
    from concourse import bacc

    dt = mybir.dt
    f32, bf16 = dt.float32, dt.bfloat16

    nc = bacc.Bacc("TRN2", target_bir_lowering=False, debug=False,
                   num_devices=N_CORES)

    xT = nc.dram_tensor("xT", [D, S], bf16, kind="ExternalInput").ap()
    wq = nc.dram_tensor("wq", [D, CW], bf16, kind="ExternalInput").ap()
    wk = nc.dram_tensor("wk", [D, CW], bf16, kind="ExternalInput").ap()
    wv = nc.dram_tensor("wv", [D, CW], bf16, kind="ExternalInput").ap()
    wo = nc.dram_tensor("wo", [D, CW], bf16, kind="ExternalInput").ap()
    cosT = nc.dram_tensor("cosT", [HD, S], bf16, kind="ExternalInput").ap()
    sinT = nc.dram_tensor("sinT", [HD, S], bf16, kind="ExternalInput").ap()
    ones = nc.dram_tensor("ones", [HD, 1], bf16, kind="ExternalInput").ap()
    masks = nc.dram_tensor("masks", [4, 128, SQT], bf16, kind="ExternalInput").ap()
    out = nc.dram_tensor("out", [S, CW], bf16, kind="ExternalOutput").ap()

    swap_mask = []
    for i in range(16):
        swap_mask += [2 * i + 1, 2 * i]

    rg = [list(range(N_CORES))]

    with tile.TileContext(nc) as tc:
        with (
            tc.tile_pool(name="consts", bufs=1) as cpool,
            tc.tile_pool(name="wqp", bufs=NK) as wqp,    # wq resident; reused by wo
            tc.tile_pool(name="wkp", bufs=NK) as wkp,    # wk resident; reused by ag
            tc.tile_pool(name="wvp", bufs=NK) as wvp,    # wv resident
            tc.tile_pool(name="xp", bufs=33) as xpool,   # x strip ring
            tc.tile_pool(name="res", bufs=1) as res,     # qrot/krot/v_sb
            tc.tile_pool(name="rope", bufs=2) as ropep,
            tc.tile_pool(name="expp", bufs=4) as expp,
            tc.tile_pool(name="accp", bufs=1) as accp,
            tc.tile_pool(name="nrm", bufs=1) as nrm,
            tc.tile_pool(name="attnsb", bufs=2) as attnsb,
            tc.tile_pool(name="psq", bufs=4, space="PSUM") as psq,
            tc.tile_pool(name="psa", bufs=4, space="PSUM") as psa,
            tc.tile_pool(name="dram", bufs=1, space="DRAM") as dram,
        ):
            # resident results of QKV+rope
            qrot = [res.tile([HD, S], bf16, name=f"qrot{h}") for h in range(HPC)]
            krot = [res.tile([HD, S], bf16, name=f"krot{h}") for h in range(HPC)]
            v_sb = [res.tile([128, CW], bf16, name=f"v{i}") for i in range(S // 128)]

            # AllGather bounce buffers (one per sq quarter)
            ag_in = [dram.tile([HPC * HD, SQT], bf16, name=f"agin{q}")
                     for q in range(NSQ)]
            ag_out = [dram.tile([D, SQT], bf16, addr_space="Shared",
                                name=f"agout{q}") for q in range(NSQ)]

            cos_sb = cpool.tile([HD, S], bf16, name="cos_sb")
            sin_sb = cpool.tile([HD, S], bf16, name="sin_sb")
            ones_sb = cpool.tile([HD, 1], bf16, name="ones_sb")
            mask_sb = [cpool.tile([128, SQT], bf16, name=f"mask{r}")
                       for r in range(4)]

            # ---- resident weights, loaded once on parallel queues ----
            # (only SP/Activation/gpsimd can issue DMAs; x streams on SP, so
            # wq rides the Activation queue and wk/consts ride gpsimd. wv is
            # issued on SP inside strip 0 after its x tiles — it is first
            # needed by strip 0's PASS C, ~66us in.)
            wq_sb, wk_sb, wv_sb = [], [], []
            for d in range(NK):
                wqt = wqp.tile([128, CW], bf16, tag="wq", name=f"wq{d}")
                if d % 2 == 1:
                    nc.gpsimd.dma_start(wqt[:], wq[d * 128:(d + 1) * 128, :])
                wq_sb.append(wqt)
            nc.gpsimd.dma_start(cos_sb[:], cosT[:])
            nc.gpsimd.dma_start(sin_sb[:], sinT[:])
            nc.gpsimd.dma_start(ones_sb[:], ones[:])
            for r in range(4):
                nc.gpsimd.dma_start(mask_sb[r][:], masks[r])

            wo_sb = []   # filled during strip 3 (reuses wqp slots)

            def emit_rope(pst, rot, sq0):
                # rot = t*cos + shuffle(t)*sin'   (sin' sign-baked)
                tbf = ropep.tile([128, SQT], bf16, tag="rbf", name="rbf")
                nc.scalar.copy(tbf[:], pst[:])          # frees the PSUM bank
                tsw = ropep.tile([128, SQT], bf16, tag="rsw", name="rsw")
                nc.vector.stream_shuffle(tsw[:], tbf[:], swap_mask)
                nc.vector.tensor_mul(tbf[:], tbf[:], cos_sb[:, sq0:sq0 + SQT])
                nc.vector.tensor_mul(tsw[:], tsw[:], sin_sb[:, sq0:sq0 + SQT])
                nc.vector.tensor_add(rot[:, sq0:sq0 + SQT], tbf[:], tsw[:])

            def strip_units(st):
                """Strip st as a list of emit units (~1us of PE work each):
                A: q heads 0-3 (streams x+wq), B: k heads 0-3, C: v. One
                PSUM bank per accumulator, all from psq; rope evacuations
                are their own units so interleaved attention work gives the
                ACT engine time to free banks before the next pass starts."""
                sq0 = st * SQT
                units = []
                x_tiles = []
                st_state = {}

                def u_head():
                    for d in range(NK):
                        xt = xpool.tile([128, SQT], bf16, tag="x",
                                        name=f"x{st}_{d}")
                        eng = nc.scalar if (st == 0 and d % 2) else nc.sync
                        eng.dma_start(xt[:], xT[d * 128:(d + 1) * 128,
                                                sq0:sq0 + SQT])
                        x_tiles.append(xt)
                    if st == 0:
                        for d in range(0, NK, 2):
                            nc.scalar.dma_start(
                                wq_sb[d][:], wq[d * 128:(d + 1) * 128, :])
                    if st == 0:
                        for d in range(NK):
                            wvt = wvp.tile([128, CW], bf16, tag="wv",
                                           name=f"wv{d}")
                            nc.sync.dma_start(wvt[:],
                                              wv[d * 128:(d + 1) * 128, :])
                            wv_sb.append(wvt)
                units.append(u_head)

                for pi, wname in enumerate(("wq", "wk")):
                    def u_pre(pi=pi):
                        wsb = wq_sb if pi == 0 else wk_sb
                        if st == 0 and pi == 1:
                            for d in range(NK):
                                wkt = wkp.tile([128, CW], bf16, tag="wk",
                                               name=f"wk{d}")
                                nc.gpsimd.dma_start(
                                    wkt[:], wk[d * 128:(d + 1) * 128, :])
                                wk_sb.append(wkt)
                        st_state[pi] = [psq.tile([128, SQT], f32, tag="b",
                                                 name=f"qk{st}_{pi}_{h}")
                                        for h in range(HPC)]
                    units.append(u_pre)
                    for d in range(NK):
                        def u_d(d=d, pi=pi):
                            wsb = wq_sb if pi == 0 else wk_sb
                            first, last = d == 0, d == NK - 1
                            xt = x_tiles[d]
                            for h in range(HPC):
                                nc.tensor.matmul(
                                    st_state[pi][h][:],
                                    wsb[d][:, h * HD:(h + 1) * HD],
                                    xt[:], start=first, stop=last)
                        units.append(u_d)
                    for h in range(HPC):
                        def u_rope(h=h, pi=pi):
                            rots = qrot if pi == 0 else krot
                            emit_rope(st_state[pi][h], rots[h], sq0)
                        units.append(u_rope)
                    if st == 3 and pi == 0:
                        def u_wo():
                            for d in range(NK):
                                wot = wqp.tile([128, CW], bf16, tag="wq",
                                               name=f"wo{d}")
                                nc.gpsimd.dma_start(
                                    wot[:], wo[d * 128:(d + 1) * 128, :])
                                wo_sb.append(wot)
                        units.append(u_wo)

                def u_vpre():
                    st_state["v"] = [psq.tile([128, CW], f32, tag="b",
                                              name=f"vps{st}_{ss}")
                                     for ss in range(4)]
                units.append(u_vpre)
                for d in range(NK):
                    def u_vd(d=d):
                        first, last = d == 0, d == NK - 1
                        for ss in range(4):
                            nc.tensor.matmul(
                                st_state["v"][ss][:],
                                x_tiles[d][:, ss * 128:(ss + 1) * 128],
                                wv_sb[d][:], start=first, stop=last)
                    units.append(u_vd)
                def u_vcopy():
                    for ss in range(4):
                        nc.scalar.copy(v_sb[st * 4 + ss][:],
                                       st_state["v"][ss][:])
                units.append(u_vcopy)
                return units

            def attn_units(sqT):
                """Attention for quarter sqT as emit units, interleaved into
                the following strip (or the output projection for sqT=3) so
                its exp/DVE load overlaps foreign matmul streams."""
                sq0 = sqT * SQT
                nblk = 4 * (sqT + 1)
                units = []
                for h in range(HPC):
                    hs = {}
                    exp_tiles = []

                    def emit_pv(j, hs=hs, h=h, exp_tiles=exp_tiles, nblk=nblk):
                        first, last = j == 0, j == nblk - 1
                        e, off = exp_tiles[j]
                        n = SQT - off
                        nc.tensor.matmul(hs["attn"][:, off:SQT],
                                         v_sb[j][:, h * HD:(h + 1) * HD],
                                         e[:, 0:n],
                                         start=first, stop=last)

                    for i in range(nblk):
                        def u_blk(i=i, h=h, hs=hs, exp_tiles=exp_tiles,
                                  nblk=nblk, emit_pv=emit_pv):
                            if i == 0:
                                hs["attn"] = psa.tile([HD, SQT], f32, tag="c",
                                                      name=f"aps{sqT}_{h}")
                                if nblk > 4:
                                    hs["acc"] = accp.tile(
                                        [128, SQT], f32, tag="acc",
                                        name=f"acc{sqT}_{h}")
                                    hs["accbf"] = accp.tile(
                                        [128, SQT], bf16, tag="accbf",
                                        bufs=1, name=f"accbf{sqT}_{h}")
                            r = i - 4 * sqT
                            off = max(0, r) * 128
                            n = SQT - off
                            sc = psa.tile([128, SQT], f32, tag="c",
                                          name=f"sc{sqT}_{h}_{i}")
                            nc.tensor.matmul(sc[:, 0:n],
                                             krot[h][:, i * 128:(i + 1) * 128],
                                             qrot[h][:, sq0 + off:sq0 + SQT],
                                             start=True, stop=True)
                            if r >= 0:
                                nc.vector.tensor_add(sc[:, 0:n], sc[:, 0:n],
                                                     mask_sb[r][:, off:SQT])
                            e = expp.tile([128, SQT], bf16, tag="e",
                                          name=f"e{sqT}_{h}_{i}")
                            nc.scalar.activation(
                                e[:, 0:n], sc[:, 0:n],
                                mybir.ActivationFunctionType.Exp, scale=SCALE)
                            if i < nblk - 4:
                                if i == 0:
                                    nc.vector.tensor_copy(hs["acc"][:], e[:])
                                else:
                                    nc.vector.tensor_add(hs["acc"][:],
                                                         hs["acc"][:], e[:])
                                if i == nblk - 5:
                                    nc.vector.tensor_copy(hs["accbf"][:],
                                                          hs["acc"][:])
                            exp_tiles.append((e, off))
                            if i >= 2:
                                emit_pv(i - 2)
                        units.append(u_blk)

                    def u_tail(h=h, hs=hs, exp_tiles=exp_tiles, nblk=nblk,
                               emit_pv=emit_pv):
                        emit_pv(nblk - 2)
                        emit_pv(nblk - 1)
                        den_ps = psa.tile([1, SQT], f32, tag="c",
                                          name=f"dps{sqT}_{h}")
                        hs["den"] = den_ps
                        if nblk > 4:
                            nc.tensor.matmul(den_ps[:], ones_sb[:],
                                             hs["accbf"][:],
                                             start=True, stop=False)
                        for jj, j in enumerate(range(nblk - 4, nblk)):
                            e, off = exp_tiles[j]
                            n = SQT - off
                            nc.tensor.matmul(den_ps[:, off:SQT], ones_sb[:],
                                             e[:, 0:n],
                                             start=(nblk == 4 and jj == 0),
                                             stop=(jj == 3))
                    units.append(u_tail)

                    def u_norm(h=h, hs=hs):
                        araw = attnsb.tile([HD, SQT], bf16, tag="a",
                                           name=f"araw{sqT}_{h}")
                        nc.vector.tensor_copy(araw[:], hs["attn"][:])
                        rec = nrm.tile([1, SQT], f32, tag="rec",
                                       name=f"rec{sqT}_{h}")
                        nc.vector.reciprocal_approx_fast(out=rec[:],
                                                         in_=hs["den"][:])
                        bc = nrm.tile([128, SQT], f32, tag="bc",
                                      name=f"bc{sqT}_{h}")
                        nc.gpsimd.partition_broadcast(bc[:], rec[:],
                                                      channels=128)
                        a_sb = attnsb.tile([HD, SQT], bf16, tag="a",
                                           name=f"asb{sqT}_{h}")
                        nc.vector.tensor_mul(a_sb[:], araw[:], bc[:])
                        nc.gpsimd.dma_start(
                            ag_in[sqT][h * HD:(h + 1) * HD, :], a_sb[:])
                    units.append(u_norm)

                def u_ag():
                    nc.gpsimd.collective_compute(
                        "AllGather", mybir.AluOpType.bypass, replica_groups=rg,
                        ins=[ag_in[sqT].opt()], outs=[ag_out[sqT].opt()])
                units.append(u_ag)
                return units

            def outproj_units(q):
                units = []
                qs = {}
                for d in range(NK):
                    def u_od(d=d, q=q, qs=qs):
                        if d == 0:
                            qs["o"] = [psq.tile([128, CW], f32, tag="b",
                                                name=f"ops{q}_{ss}")
                                       for ss in range(4)]
                        agt = wkp.tile([128, SQT], bf16, tag="wk",
                                       name=f"agt{q}_{d}")
                        nc.sync.dma_start(agt[:],
                                          ag_out[q][d * 128:(d + 1) * 128, :])
                        first, last = d == 0, d == NK - 1
                        for ss in range(4):
                            nc.tensor.matmul(
                                qs["o"][ss][:],
                                agt[:, ss * 128:(ss + 1) * 128],
                                wo_sb[d][:], start=first, stop=last)
                    units.append(u_od)
                def u_ost(q=q, qs=qs):
                    eng = nc.sync if q == 3 else nc.gpsimd
                    for ss in range(4):
                        o = attnsb.tile([128, CW], bf16, tag="a",
                                        name=f"o{q}_{ss}")
                        nc.scalar.copy(o[:], qs["o"][ss][:])
                        eng.dma_start(
                            out[q * SQT + ss * 128:q * SQT + (ss + 1) * 128,
                                :], o[:])
                units.append(u_ost)
                return units

            def interleave(primary, secondary, frac=1.0):
                # drain `secondary` within the first `frac` of `primary`
                n, m = len(primary), len(secondary)
                j = 0
                for i, u in enumerate(primary):
                    u()
                    target = min(m, int((i + 1) * m / (n * frac)))
                    while j < target:
                        secondary[j]()
                        j += 1
                while j < m:
                    secondary[j]()
                    j += 1

            for st in range(NSQ):
                su = strip_units(st)
                au = attn_units(st - 1) if st >= 1 else []
                interleave(su, au)
            # attention of the last strip interleaves into outproj q0-q2;
            # its AllGather completes under outproj q2's matmuls
            ou_pre = []
            for q in range(3):
                ou_pre += outproj_units(q)
            au3 = attn_units(NSQ - 1)
            # prime attention 3 (its inputs are long ready; the first ag_out
            # loads pay DMA latency) then drain it within ~60% of outproj
            # q0-q2 so AllGather(3) completes well before outproj q3
            for u in au3[:2]:
                u()
            interleave(ou_pre, au3[2:], frac=0.5)
            for u in outproj_units(3):
                u()

    nc.compile()
    return nc


def _prep_inputs(x, wq, wk, wv, wo, freqs_cos, freqs_sin, mask):
    bf16 = ml_dtypes.bfloat16
    x2 = np.asarray(x, dtype=np.float32).reshape(S, D)
    xT = np.ascontiguousarray(x2.T).astype(bf16)
    cosT = np.repeat(np.asarray(freqs_cos, np.float32).T, 2, axis=0)
    sinT = np.repeat(np.asarray(freqs_sin, np.float32).T, 2, axis=0).copy()
    sinT[0::2] *= -1.0
    cosT = np.ascontiguousarray(cosT).astype(bf16)
    sinT = np.ascontiguousarray(sinT).astype(bf16)
    m2 = np.asarray(mask, np.float32).reshape(S, S)
    masks = np.stack([np.ascontiguousarray(m2[0:SQT, r * 128:(r + 1) * 128].T)
                      for r in range(4)]).astype(bf16)  # [4, 128, 512]
    in_maps = []
    for c in range(N_CORES):
        cols = slice(c * CW, (c + 1) * CW)
        in_maps.append({
            "xT": xT,
            "wq": np.ascontiguousarray(np.asarray(wq, np.float32)[:, cols]).astype(bf16),
            "wk": np.ascontiguousarray(np.asarray(wk, np.float32)[:, cols]).astype(bf16),
            "wv": np.ascontiguousarray(np.asarray(wv, np.float32)[:, cols]).astype(bf16),
            "wo": np.ascontiguousarray(np.asarray(wo, np.float32)[:, cols]).astype(bf16),
            "cosT": cosT,
            "ones": np.ones((HD, 1), bf16),
            "sinT": sinT,
            "masks": masks,
        })
    return in_maps


def kernel(x, wq, wk, wv, wo, freqs_cos, freqs_sin, mask):
    global LAST_RESULT
    from concourse.bass_utils import run_bass_kernel_spmd

    if "nc" not in _CACHE:
        _CACHE["nc"] = _build()
    nc = _CACHE["nc"]
    in_maps = _prep_inputs(x, wq, wk, wv, wo, freqs_cos, freqs_sin, mask)
    res = run_bass_kernel_spmd(nc, in_maps, core_ids=list(range(N_CORES)))
    LAST_RESULT = res
    out = np.concatenate([res.results[c]["out"].astype(np.float32)
                          for c in range(N_CORES)], axis=1)
    return out.reshape(B, S, D)
